# revision 1
# baseline (speedup 1.0000x reference)
"""Trainium2 Bass kernel for banded (sliding-window) single-head attention.

Problem (hardcoded):
    x     [256, 256, 768] f32   (batch, tokens, dim)
    w_qkv [768, 192]      f32
    w_out [64, 768]       f32
    b_out [768]           f32
    y = (softmax(band_mask(q k^T / 8)) v) @ w_out + b_out,  band |i-j| < 32

Strategy: pure data parallel over batch (32 batches/core on 8 cores).

Per-core kernel design (all-fp16 operands, fp32 PSUM accumulation):
  - x arrives as two fp16 planes (hi = fp16(x), lo = fp16(x - hi)); together
    they carry ~22 mantissa bits, and total input bytes equal the original
    fp32.  The 2-byte DMA xbar transpose loads both planes transposed
    (partition = feature dim) in one descriptor burst each; the planes are
    consumed directly by the QKV matmuls as two accumulating products
    (W.T @ x_hi + W.T @ x_lo), so no on-chip reconstruction is needed.
    (The 4-byte xbar path is unsupported on TRN2 and fp32/f32r engine
    round-trips proved unreliable, so fp16 is both fastest and safest.)
  - qkT [128(q|k), tok] and vT [64, tok] via per-chunk accumulation.
  - Per batch (256 tokens):
      v natural via 2 PE transposes of vT, + ones column -> v_aug [128, 65]
      scoresT[j, i] = kT[:, jc]^T @ qT  (two j-chunks of 128)
      expT = exp(scoresT / 8) * band_mask  (ACT exp -> fp16, DVE mask mul)
      outT_aug [65, i] = sum_jc v_aug[jc]^T @ expT[jc]  (row 64 = softmax sums)
      scale by 1/sums broadcast over partitions -> row 64 becomes ones
      final [i-chunk, 768] = outT_aug_scaled[:, ic]^T @ [w_out; b_out]
        (the ones row times b_out applies the bias inside the matmul)
"""

import numpy as np

import concourse.bass as bass
import concourse.mybir as mybir
import concourse.tile as tile
from concourse import bacc
from concourse import bass_utils

F32 = mybir.dt.float32
F16 = mybir.dt.float16

B, N, D, DH = 256, 256, 768, 64
SA = 32                       # band half-width: |i-j| < SA
NCORES = 8
BLOC = B // NCORES            # batches per core
TOK_FULL = BLOC * N           # tokens per core (8192)
PT = 512                      # tokens per pipeline tile (2 batches)
NC_CHUNKS = D // 128          # 6 contraction chunks



def build_body(tc, x_blk, w_qkv, w_out, b_out, y, tok, ctx, dbg=None):
    nc = tc.nc
    npt = tok // PT
    nbatch_pt = PT // N       # batches per ptile (2)

    const = ctx.enter_context(tc.tile_pool(name="const", bufs=1))
    xplane_pool = ctx.enter_context(tc.tile_pool(name="xplane", bufs=4))
    qkv_pool = ctx.enter_context(tc.tile_pool(name="qkv", bufs=2))
    vaug_pool = ctx.enter_context(tc.tile_pool(name="vaug", bufs=3))
    exp_pool = ctx.enter_context(tc.tile_pool(name="exp", bufs=3))
    osc_pool = ctx.enter_context(tc.tile_pool(name="osc", bufs=4))
    small_pool = ctx.enter_context(tc.tile_pool(name="small", bufs=2))
    y_pool = ctx.enter_context(tc.tile_pool(name="ysb", bufs=2))

    ps_proj = ctx.enter_context(tc.tile_pool(name="psproj", bufs=2, space="PSUM"))
    ps_mm = ctx.enter_context(tc.tile_pool(name="psmm", bufs=4, space="PSUM"))
    ps_o = ctx.enter_context(tc.tile_pool(name="pso", bufs=2, space="PSUM"))

    # ---- constants ----
    # w_qkv rearranged so chunk c holds rows [c*128, (c+1)*128)
    wq_sb = const.tile([128, NC_CHUNKS, 192], F16)
    nc.sync.dma_start(out=wq_sb[:], in_=w_qkv.rearrange("(c p) e -> p c e", p=128))

    # [w_out; b_out] as a 65-row augmented matrix
    waug = const.tile([65, D], F16)
    nc.sync.dma_start(out=waug[0:64, :], in_=w_out[:, :])
    nc.sync.dma_start(out=waug[64:65, :], in_=b_out.unsqueeze(0))

    # band masks for the two j-chunks: mask[jc][j, i] = 1 if |i - (jc*128+j)| < SA
    maskt_f32 = const.tile([128, 2, N], F32)
    nc.gpsimd.memset(maskt_f32[:], 1.0)
    for jc in range(2):
        j0 = jc * 128
        # keep where (SA-1) + i - (j0 + p) >= 0
        nc.gpsimd.affine_select(
            out=maskt_f32[:, jc, :], in_=maskt_f32[:, jc, :],
            compare_op=mybir.AluOpType.is_ge, fill=0.0,
            base=SA - 1 - j0, channel_multiplier=-1, pattern=[[1, N]],
        )
        # keep where (SA-1) + (j0 + p) - i >= 0
        nc.gpsimd.affine_select(
            out=maskt_f32[:, jc, :], in_=maskt_f32[:, jc, :],
            compare_op=mybir.AluOpType.is_ge, fill=0.0,
            base=SA - 1 + j0, channel_multiplier=1, pattern=[[-1, N]],
        )
    maskt = const.tile([128, 2, N], F16)
    nc.scalar.copy(maskt[:], maskt_f32[:])

    # identity for PE transposes of vT slices
    ident_f32 = const.tile([64, 64], F32)
    nc.gpsimd.memset(ident_f32[:], 0.0)
    nc.gpsimd.affine_select(
        out=ident_f32[:], in_=ident_f32[:],
        compare_op=mybir.AluOpType.not_equal, fill=1.0,
        base=0, channel_multiplier=1, pattern=[[-1, 64]],
    )
    ident = const.tile([64, 64], F16)
    nc.scalar.copy(ident[:], ident_f32[:])

    ones128 = const.tile([128, 1], F16)
    nc.vector.memset(ones128[:], 1.0)
    e65 = const.tile([65, 1], F16)
    nc.vector.memset(e65[:], 0.0)
    nc.vector.memset(e65[64:65, :], 1.0)

    # ---- main pipeline over ptiles of PT tokens ----
    for pt in range(npt):
        t_lo = pt * PT
        # transposed fp16 planes, one xbar DMA from a contiguous 1.5 MB
        # block: out[p, cc, t] = x[t_lo+t, cc*128+p]; cc 0-5 hi, 6-11 lo
        xp = xplane_pool.tile([128, 2 * NC_CHUNKS, PT], F16, tag="xp")
        nrows = 2 * NC_CHUNKS * PT
        nc.sync.dma_start(out=xp[:], in_=x_blk[pt * nrows:(pt + 1) * nrows, :],
                          transpose=True)

        # qkT: [128(e = q|k), PT] = sum_c W_qk[c].T @ (x_hi[c] + x_lo[c])
        qk_ps = ps_proj.tile([128, PT], F32, tag="proj")
        for c in range(NC_CHUNKS):
            nc.tensor.matmul(
                qk_ps[:], lhsT=wq_sb[:, c, 0:128], rhs=xp[:, c, :],
                start=(c == 0), stop=False,
            )
            nc.tensor.matmul(
                qk_ps[:], lhsT=wq_sb[:, c, 0:128], rhs=xp[:, NC_CHUNKS + c, :],
                start=False, stop=(c == NC_CHUNKS - 1),
            )
        qT = qkv_pool.tile([64, PT], F16, tag="qT")
        kT = qkv_pool.tile([64, PT], F16, tag="kT")
        nc.scalar.copy(qT[:], qk_ps[0:64, :])
        nc.vector.tensor_copy(kT[:], qk_ps[64:128, :])

        # vT: [64, PT]
        v_ps = ps_proj.tile([64, PT], F32, tag="proj")
        for c in range(NC_CHUNKS):
            nc.tensor.matmul(
                v_ps[:], lhsT=wq_sb[:, c, 128:192], rhs=xp[:, c, :],
                start=(c == 0), stop=False,
            )
            nc.tensor.matmul(
                v_ps[:], lhsT=wq_sb[:, c, 128:192], rhs=xp[:, NC_CHUNKS + c, :],
                start=False, stop=(c == NC_CHUNKS - 1),
            )
        vT = qkv_pool.tile([64, PT], F16, tag="vT")
        nc.scalar.copy(vT[:], v_ps[:])



        if dbg is not None and pt == 0:
            nc.sync.dma_start(out=dbg["qT"][:, :], in_=qT[:])
            nc.sync.dma_start(out=dbg["kT"][:, :], in_=kT[:])
            nc.sync.dma_start(out=dbg["vT"][:, :], in_=vT[:])

        y_sb = y_pool.tile([128, PT // 128, D], F32)

        # ---- stage-interleaved across the batches of this ptile, so the
        # per-batch serial chains (scores->exp->mask->pv->normalize->final)
        # overlap instead of concatenating their latencies ----
        vaugs, pexps, oscs, o_pss = [], [], [], []

        for bb in range(nbatch_pt):
            t0 = bb * N
            vt_ps = ps_mm.tile([128, 2, 64], F16, tag="mm")
            vaug = vaug_pool.tile([128, 2, 65], F16)
            for jc in range(2):
                nc.tensor.transpose(
                    vt_ps[:, jc, :], vT[:, t0 + jc * 128: t0 + (jc + 1) * 128],
                    ident[:],
                )
            nc.vector.tensor_copy(vaug[:, :, 0:64], vt_ps[:, :, :])
            nc.vector.memset(vaug[:, :, 64:65], 1.0)
            vaugs.append(vaug)

        sc_list = []
        for bb in range(nbatch_pt):
            t0 = bb * N
            for jc in range(2):
                sc_ps = ps_mm.tile([128, N], F32, tag="mm")
                nc.tensor.matmul(
                    sc_ps[:], lhsT=kT[:, t0 + jc * 128: t0 + (jc + 1) * 128],
                    rhs=qT[:, t0: t0 + N], start=True, stop=True,
                )
                sc_list.append(sc_ps)

        for bb in range(nbatch_pt):
            pexp = exp_pool.tile([128, 2, N], F16)
            for jc in range(2):
                nc.scalar.activation(
                    pexp[:, jc, :], sc_list[bb * 2 + jc][:],
                    mybir.ActivationFunctionType.Exp, scale=float(DH) ** -0.5,
                )
            pexps.append(pexp)

        for bb in range(nbatch_pt):
            pexp = pexps[bb]
            # split the band-mask multiplies across DVE and GpSimd
            nc.gpsimd.tensor_mul(pexp[:, 0, :], pexp[:, 0, :], maskt[:, 0, :])
            nc.gpsimd.tensor_mul(pexp[:, 1, :], pexp[:, 1, :], maskt[:, 1, :])

        for bb in range(nbatch_pt):
            o_ps = ps_o.tile([65, N], F32, tag="o")
            for jc in range(2):
                nc.tensor.matmul(
                    o_ps[:], lhsT=vaugs[bb][:, jc, :], rhs=pexps[bb][:, jc, :],
                    start=(jc == 0), stop=(jc == 1),
                )
            o_pss.append(o_ps)

        for bb in range(nbatch_pt):
            osc = osc_pool.tile([65, N], F16)
            nc.scalar.copy(osc[:], o_pss[bb][:])
            oscs.append(osc)

        # per-token sums column: osc row 64 selected by a trivial N=1 matmul
        # (osc.T @ e65), then a 128-lane-parallel reciprocal
        recip_cols = []
        for bb in range(nbatch_pt):
            scol_ps = ps_mm.tile([128, 2], F32, tag="mm")
            for ic in range(2):
                nc.tensor.matmul(
                    scol_ps[:, ic:ic + 1],
                    lhsT=oscs[bb][:, ic * 128:(ic + 1) * 128], rhs=e65[:],
                    start=True, stop=True,
                )
            rcol = small_pool.tile([128, 2], F32, tag="rcol")
            nc.vector.reciprocal(rcol[:], scol_ps[:])
            recip_cols.append(rcol)

        for bb in range(nbatch_pt):
            for ic in range(2):
                rcol = recip_cols[bb][:, ic:ic + 1]
                f_ps = ps_mm.tile([128, 384], F32, tag="mm")
                f_ps2 = ps_mm.tile([128, 384], F32, tag="mm")
                nc.tensor.matmul(
                    f_ps[:], lhsT=oscs[bb][:, ic * 128:(ic + 1) * 128],
                    rhs=waug[:, 0:384], start=True, stop=True,
                )
                nc.tensor.matmul(
                    f_ps2[:], lhsT=oscs[bb][:, ic * 128:(ic + 1) * 128],
                    rhs=waug[:, 384:768], start=True, stop=True,
                )
                # PSUM->SBUF copies with the softmax normalization fused in
                # as a per-partition (per-token) scale
                nc.scalar.activation(
                    y_sb[:, bb * 2 + ic, 0:384], f_ps[:],
                    mybir.ActivationFunctionType.Copy, scale=rcol,
                )
                nc.vector.tensor_scalar_mul(
                    y_sb[:, bb * 2 + ic, 384:768], f_ps2[:], rcol,
                )

        nc.scalar.dma_start(
            out=y[t_lo:t_lo + PT, :].rearrange("(ic p) d -> p ic d", p=128),
            in_=y_sb[:],
        )



def build_nc(tok=TOK_FULL, debug_taps=False):
    nc = bacc.Bacc("TRN2", target_bir_lowering=False, debug=False)
    # x planes hi|lo, rows blocked chunk-major per ptile (row = cc*PT + t)
    # so the xbar transpose reads 4KB-contiguous tiles (HW-verified mapping:
    # out[p, cc, t] = in[cc*PT + t, p]; note CoreSim models this t-major
    # instead -- the sim harness feeds the t-major equivalent)
    x_blk = nc.dram_tensor("x_blk", [tok * 2 * D // 128, 128], F16,
                           kind="ExternalInput").ap()
    w_qkv = nc.dram_tensor("w_qkv", [D, 3 * DH], F16, kind="ExternalInput").ap()
    w_out = nc.dram_tensor("w_out", [DH, D], F16, kind="ExternalInput").ap()
    b_out = nc.dram_tensor("b_out", [D], F16, kind="ExternalInput").ap()
    y = nc.dram_tensor("y", [tok, D], F32, kind="ExternalOutput").ap()

    dbg = None
    if debug_taps:
        dbg = {
            "qT": nc.dram_tensor("dbg_qT", [64, PT], F16,
                                 kind="ExternalOutput").ap(),
            "kT": nc.dram_tensor("dbg_kT", [64, PT], F16,
                                 kind="ExternalOutput").ap(),
            "vT": nc.dram_tensor("dbg_vT", [64, PT], F16,
                                 kind="ExternalOutput").ap(),
            "vaug": nc.dram_tensor("dbg_vaug", [128, 2 * 65], F16,
                                   kind="ExternalOutput").ap(),
            "pexp": nc.dram_tensor("dbg_pexp", [128, 2 * N], F16,
                                   kind="ExternalOutput").ap(),
            "osc": nc.dram_tensor("dbg_osc", [65, N], F16,
                                  kind="ExternalOutput").ap(),
        }

    from contextlib import ExitStack
    with tile.TileContext(nc) as tc:
        with ExitStack() as ctx:
            build_body(tc, x_blk, w_qkv, w_out, b_out, y, tok, ctx, dbg=dbg)
    nc.compile()
    return nc


def split_f16(x):
    hi = x.astype(np.float16)
    lo = (x - hi.astype(np.float32)).astype(np.float16)
    return hi, lo


def make_in_maps(x, w_qkv, w_out, b_out):
    in_maps = []
    w_qkv16 = np.asarray(w_qkv, dtype=np.float16)
    w_out16 = np.asarray(w_out, dtype=np.float16)
    b_out16 = np.asarray(b_out, dtype=np.float16)
    for c in range(NCORES):
        xc = np.ascontiguousarray(
            np.asarray(x)[c * BLOC:(c + 1) * BLOC].reshape(TOK_FULL, D),
            dtype=np.float32,
        )
        hi, lo = split_f16(xc)
        pair = np.concatenate([hi, lo], axis=1)          # [tok, 1536]
        npt = xc.shape[0] // PT
        blk = np.ascontiguousarray(
            pair.reshape(npt, PT, 12, 128).transpose(0, 2, 1, 3)
        ).reshape(-1, 128)
        in_maps.append({
            "x_blk": blk,
            "w_qkv": w_qkv16, "w_out": w_out16, "b_out": b_out16,
        })
    return in_maps


_NC_CACHE = {}


def run(x, w_qkv, w_out, b_out, trace=False, **trace_kwargs):
    if "nc" not in _NC_CACHE:
        _NC_CACHE["nc"] = build_nc()
    nc = _NC_CACHE["nc"]
    in_maps = make_in_maps(x, w_qkv, w_out, b_out)
    res = bass_utils.run_bass_kernel_spmd(
        nc, in_maps, core_ids=list(range(NCORES)), trace=trace, **trace_kwargs
    )
    y = np.concatenate(
        [res.results[c]["y"].reshape(BLOC, N, D) for c in range(NCORES)], axis=0
    )
    return y.astype(np.float32), res


def kernel(x, w_qkv, w_out, b_out):
    y, _ = run(np.asarray(x), np.asarray(w_qkv), np.asarray(w_out),
               np.asarray(b_out))
    return y

